# revision 27
# baseline (speedup 1.0000x reference)
"""Trainium2 Bass kernel for nn_Decoder (moe_routing, 4-species expert decoder).

Reference semantics (per species i, m = 4096 entries; only the first 512
decoded rows are ever read because decoded[bi, gi] indexes rows with *cell*
ids < 512):

    bi   = batch_idx[i*m:(i+1)*m]            # cell ids < 512
    gi   = gene_idx[i*m:(i+1)*m]
    comb = concat(z[i][:512], global_latent[bi[:512]])       # [512, 512]
    h1   = relu(comb @ W1[i] + b1[i])                        # [512, 1024]
    h2   = relu(h1 @ W2[i] + b2[i])                          # [512, 1024]
    out[e] = softplus(h2[bi[e]] . W3[i][:, gi[e]] + b3[i][gi[e]])

Sharding: expert-parallel x row-parallel.  Core c handles species c//2 and
MLP rows [256*(c%2), 256*(c%2+1)) plus every entry whose cell id falls in
that row range, so the two cores of a species split the MLP instead of
duplicating it.  The host pre-gathers everything indexable at input-prep
time: comb^T directly (z slice + global_latent[bi]), the W3 columns each
entry needs (w3gT, in [h, entry] layout), and b3[gi].  The per-entry dot
then runs on the PE: per 128-entry chunk, Q[e, c] = w3col_e . h2T[:, c]
(8 accumulating matmuls against the resident h2T), after which a one-hot
mask multiply + 128-wide reduce on the DVE picks Q[e, bi[e]].  h2 is kept
in [h, row] layout so both biases are per-partition activation biases.
Math in bf16 with f32 accumulation; no device gathers, no DRAM spills.
"""

import os
import sys

import numpy as np

for _p in ("/root/.axon_site/_ro/trn_rl_repo", "/opt/trn_rl_repo"):
    if os.path.isdir(_p) and _p not in sys.path:
        sys.path.append(_p)

import ml_dtypes

BF = ml_dtypes.bfloat16

N_SPECIES = 4
NNZ = 16384
N_CELLS = 512
L = 256          # latent
H = 1024         # hidden
G = 20000        # genes
M = NNZ // N_SPECIES   # 4096 entries per species
R = 512          # decoded rows actually used per species
RC = 256         # rows per core (half of R)
N_CORES = 8

_NC = {}              # CHT -> compiled Bass module
LAST_RESULTS = None   # BassKernelResults of the last run (for profiling)


def _build_nc(CHT):
    """CHT = 128-entry chunks per 128-row group (2 groups per core)."""
    from contextlib import ExitStack

    import concourse.bacc as bacc
    import concourse.mybir as mybir
    import concourse.tile as tile

    F32 = mybir.dt.float32
    BF16 = mybir.dt.bfloat16
    AF = mybir.ActivationFunctionType
    OP = mybir.AluOpType

    NT = 2 * CHT
    CB = 4 * RC          # combT cols in the bf16 pack
    OH0 = CB             # ohm offset in the bf16 pack

    nc = bacc.Bacc(None, target_bir_lowering=False)

    # pf32 packs b1s [128,8] | b2s [128,8] | b3g [128,NT].
    combT = nc.dram_tensor("combT", [128, CB], BF16, kind="ExternalInput")
    ohm = nc.dram_tensor("ohm", [128, NT * 128], BF16, kind="ExternalInput")
    pf32 = nc.dram_tensor("pf32", [128, 16 + NT], F32, kind="ExternalInput")
    w1r = nc.dram_tensor("w1r", [128, 8, 4, 128], BF16, kind="ExternalInput")
    w2r = nc.dram_tensor("w2r", [128, 8, 8, 128], BF16, kind="ExternalInput")
    w3gT = nc.dram_tensor("w3gT", [128, NT, 8, 128], BF16, kind="ExternalInput")
    out = nc.dram_tensor("out", [128, NT], F32, kind="ExternalOutput")

    with tile.TileContext(nc) as tc, ExitStack() as ctx:
        const = ctx.enter_context(tc.tile_pool(name="const", bufs=1))
        work = ctx.enter_context(tc.tile_pool(name="work", bufs=1))
        scrp = ctx.enter_context(tc.tile_pool(name="scr", bufs=2))
        pmm = ctx.enter_context(tc.tile_pool(name="pmm", bufs=2, space="PSUM"))
        pq = ctx.enter_context(tc.tile_pool(name="pq", bufs=4, space="PSUM"))

        # All loads ride the sync ring in strict consumption-priority order,
        # each piece its own tile so compute can start the moment its piece
        # lands: combT, w1 halves, biases, w2 halves, masks, then the W3
        # table streamed in dot order.  (The scalar ring stays free so relus
        # are never stuck behind DIRECT2D issue.)
        # Loads ride BOTH DMA rings (sync + scalar) so two SW-DGE engines
        # feed the 16 HW queues, alternating in consumption-priority order.
        # W3 pieces stream t-major so each 128-entry chunk's dot can run as
        # soon as its piece lands; the last pieces taper down to one chunk to
        # minimize the post-DMA tail.
        combT_s = const.tile([128, CB], BF16, tag="combT")
        nc.sync.dma_start(combT_s[:], combT[:])
        pf_s = const.tile([128, 16 + NT], F32, tag="pf")
        nc.sync.dma_start(pf_s[:], pf32[:])
        w1_t = []
        w1_dmas = []
        for h in range(4):
            w1_t.append(const.tile([128, 2, 4, 128], BF16, name=f"w1{h}", tag=f"w1{h}"))
            w1_dmas.append(nc.sync.dma_start(
                w1_t[h][:], w1r[:, 2 * h : 2 * (h + 1), :, :]))
        w2_t = []
        for h in range(4):
            w2_t.append(const.tile([128, 2, 8, 128], BF16, name=f"w2{h}", tag=f"w2{h}"))
            nc.sync.dma_start(w2_t[h][:], w2r[:, 2 * h : 2 * (h + 1), :, :])
        ohm_s = const.tile([128, NT * 128], BF16, tag="ohm")
        nc.sync.dma_start(ohm_s[:], ohm[:])
        pieces = []
        left = NT
        while left > 3:
            pieces.append(3)
            left -= 3
        pieces += [2, 1] if left == 3 else [left]
        assert sum(pieces) == NT

        # PE warmup while the weight DMA is in flight: throwaway matmuls keep
        # TensorE busy from the start so the activity monitor is in fast mode
        # (a cold PE runs at half rate) by the time h1 starts.
        wu = const.tile([128, RC], BF16, tag="wu")
        nc.gpsimd.memset(wu[:], 0)
        one1 = const.tile([128, 1], F32, tag="one1")
        nc.gpsimd.memset(one1[:], 1.0)
        for w in range(6):
            pw = pq.tile([128, 128], F32, tag="q")
            nc.tensor.matmul(pw[:], wu[:, 0:128], wu[:, 0:128], start=True,
                             stop=True)
        # Preload the Exp/Ln activation tables during the DMA wait so the
        # softplus tail doesn't eat a 1.3us ACT_TABLE_LOAD.
        wrm = work.tile([128, 2], F32, tag="wrm")
        nc.scalar.activation(wrm[:, 0:1], one1[:], AF.Exp)
        nc.scalar.activation(wrm[:, 1:2], one1[:], AF.Ln)

        # h1T[h, row] = relu(W1_mt.T @ combT + b1): per-partition bias.
        h1T = work.tile([128, 8, RC], BF16, tag="h1T")
        for mt in range(8):
            ps = pmm.tile([128, RC], F32, tag="ps")
            for kt in range(4):
                mm = nc.tensor.matmul(
                    ps[:],
                    w1_t[mt // 2][:, mt % 2, kt, :],
                    combT_s[:, kt * RC : (kt + 1) * RC],
                    start=(kt == 0),
                    stop=(kt == 3),
                )
            nc.scalar.activation(
                h1T[:, mt, :], ps[:], AF.Relu, bias=pf_s[:, mt : mt + 1]
            )

        # The W3 stream rides the otherwise-idle gpsimd ring (a second SW-DGE
        # feeder), held until h1's last matmul retires -- i.e. until the w1/
        # combT prefix has actually been consumed -- so the prefix transfers
        # keep full queue bandwidth.  (A dep on the w1 DMA itself would only
        # order descriptor *issue*, and the W3 flood would starve the prefix.)
        from concourse.tile_rust import add_dep_helper

        w3_t = []   # per chunk t -> (tile, col offset)
        pi = 0
        first_w3 = None
        for np_ in pieces:
            tw = const.tile([128, np_, 8, 128], BF16, name=f"w3p{pi}",
                            tag=f"w3p{pi}")
            d = nc.gpsimd.dma_start(tw[:], w3gT[:, pi : pi + np_, :, :])
            if first_w3 is None:
                first_w3 = d
                add_dep_helper(d.ins, mm.ins, sync=True,
                               reason="w3 stream yields to the w1/h1 prefix")
            for u in range(np_):
                w3_t.append((tw, u))
            pi += np_

        # h2T[h, row] = relu(W2_ht.T @ h1T + b2): per-partition bias.
        h2T = work.tile([128, 8, RC], BF16, tag="h2T")
        for ht in range(8):
            ps = pmm.tile([128, RC], F32, tag="ps")
            for k2 in range(8):
                nc.tensor.matmul(
                    ps[:],
                    w2_t[ht // 2][:, ht % 2, k2, :],
                    h1T[:, k2, :],
                    start=(k2 == 0),
                    stop=(k2 == 7),
                )
            nc.scalar.activation(
                h2T[:, ht, :], ps[:], AF.Relu, bias=pf_s[:, 8 + ht : 9 + ht]
            )

        # Per 128-entry chunk t (cell group g = t // CHT):
        #   Q[e, c] = sum_h w3gT[h, e] * h2T[h, c]     (8 matmuls, k-tiled)
        #   dots[e] = sum_c (Q[e,c] + b3[e]) * ohm[e,c] = Q[e, bi_e] + b3[e]
        # (each real mask row sums to 1, pad rows to 0) in one fused DVE op.
        dots = work.tile([128, NT], F32, tag="dots")
        for t in range(NT):
            g = t // CHT
            tw, u = w3_t[t]
            q = pq.tile([128, 128], F32, tag="q")
            for kt in range(8):
                nc.tensor.matmul(
                    q[:],
                    tw[:, u, kt, :],
                    h2T[:, kt, g * 128 : (g + 1) * 128],
                    start=(kt == 0),
                    stop=(kt == 7),
                )
            scr = scrp.tile([128, 128], BF16, tag="scr")
            nc.vector.scalar_tensor_tensor(
                out=scr[:],
                in0=q[:],
                scalar=pf_s[:, 16 + t : 17 + t],
                in1=ohm_s[:, t * 128 : (t + 1) * 128],
                op0=OP.add,
                op1=OP.mult,
                accum_out=dots[:, t : t + 1],
            )

        # softplus(x) = ln(e^x + 1) via the Exp and Ln activation tables,
        # the +1 folded into Ln's per-partition bias; both ops on Scalar.
        u = work.tile([128, NT], F32, tag="u")
        l = work.tile([128, NT], F32, tag="l")
        nc.scalar.activation(u[:], dots[:], AF.Exp)
        nc.scalar.activation(l[:], u[:], AF.Ln, bias=one1[:])
        nc.scalar.dma_start(out[:], l[:])

    nc.finalize()
    return nc


def _get_nc(CHT):
    if CHT not in _NC:
        _NC[CHT] = _build_nc(CHT)
    return _NC[CHT]


def _prep_core_inputs(c, CHT, batch_idx, gene_idx, global_latent, z, W1, b1,
                      W2, b2, W3, b3):
    """Build the device input map for core c plus the slot->global-entry map
    used to assemble the output (slot s = t*128 + p; -1 = padding)."""
    i, j = c // 2, c % 2
    NT = 2 * CHT
    CB = 4 * RC
    bi_sp = np.asarray(batch_idx[i * M : (i + 1) * M], dtype=np.int64)
    gi_sp = np.asarray(gene_idx[i * M : (i + 1) * M], dtype=np.int64)
    bi512 = np.asarray(batch_idx[i * M : i * M + R], dtype=np.int64)

    slot_entry = np.full(NT * 128, -1, dtype=np.int64)
    gi_slots = np.zeros(NT * 128, dtype=np.int64)
    b3_slots = np.zeros(NT * 128, dtype=np.float32)
    ohm = np.zeros((128, NT * 128), dtype=BF)  # [entry partition, t*128 + c]
    for g in range(2):
        eg = np.nonzero(bi_sp // 128 == 2 * j + g)[0]
        assert len(eg) <= CHT * 128
        ss = np.arange(len(eg))
        tt = g * CHT + ss // 128
        pp = ss % 128
        slots = tt * 128 + pp
        slot_entry[slots] = i * M + eg
        gi_slots[slots] = gi_sp[eg]
        b3_slots[slots] = b3[i][gi_sp[eg]]
        ohm[pp, tt * 128 + (bi_sp[eg] % 128)] = 1

    # comb^T for this core's rows: feature f x row r' (r = RC*j + r')
    rows = slice(RC * j, RC * (j + 1))
    comb = np.concatenate(
        [z[i][rows], global_latent[bi512[rows]]], axis=1)  # [RC, 2L]
    combT = comb.T.astype(BF)  # [512 feat, RC]
    # col layout: kt*RC + r' with feature f = kt*128 + p
    combT_pk = np.ascontiguousarray(
        combT.reshape(4, 128, RC).transpose(1, 0, 2).reshape(128, CB))

    pf32 = np.empty((128, 16 + NT), dtype=np.float32)
    pf32[:, 0:8] = b1[i].reshape(8, 128).T
    pf32[:, 8:16] = b2[i].reshape(8, 128).T
    pf32[:, 16:] = b3_slots.reshape(NT, 128).T

    # W3 columns for each slot in [h, entry] layout: w3gT[p, t, kt, q] =
    # W3[i][kt*128+p, gi_slots[t*128+q]]
    w3cols = W3[i][:, gi_slots].astype(BF)  # [H, NT*128]
    w3gT = np.ascontiguousarray(
        w3cols.reshape(8, 128, NT, 128).transpose(1, 2, 0, 3))

    in_map = {
        "combT": combT_pk,
        "ohm": ohm,
        "pf32": pf32,
        "w1r": np.ascontiguousarray(
            W1[i].reshape(4, 128, 8, 128).transpose(1, 2, 0, 3)).astype(BF),
        "w2r": np.ascontiguousarray(
            W2[i].reshape(8, 128, 8, 128).transpose(1, 2, 0, 3)).astype(BF),
        "w3gT": w3gT,
    }
    return in_map, slot_entry


def kernel(values, batch_idx, gene_idx, global_latent, z, W1, b1, W2, b2, W3,
           b3):
    global LAST_RESULTS
    from concourse.bass_utils import run_bass_kernel_spmd

    batch_idx = np.asarray(batch_idx)
    gene_idx = np.asarray(gene_idx)
    global_latent = np.asarray(global_latent, dtype=np.float32)
    z = np.asarray(z, dtype=np.float32)
    W1 = np.asarray(W1, dtype=np.float32)
    b1 = np.asarray(b1, dtype=np.float32)
    W2 = np.asarray(W2, dtype=np.float32)
    b2 = np.asarray(b2, dtype=np.float32)
    W3 = np.asarray(W3, dtype=np.float32)
    b3 = np.asarray(b3, dtype=np.float32)

    # Chunks per 128-row group: sized to the largest group so padding is <1
    # chunk; compiled kernels are cached per CHT.
    counts = np.bincount(np.asarray(batch_idx, dtype=np.int64) // 128
                         + 4 * (np.arange(NNZ) // M), minlength=16)
    CHT = int(-(-counts.max() // 128))
    nc = _get_nc(CHT)

    in_maps, slot_maps = [], []
    for c in range(N_CORES):
        im, se = _prep_core_inputs(c, CHT, batch_idx, gene_idx, global_latent,
                                   z, W1, b1, W2, b2, W3, b3)
        in_maps.append(im)
        slot_maps.append(se)

    LAST_RESULTS = run_bass_kernel_spmd(nc, in_maps, core_ids=list(range(N_CORES)))

    output = np.zeros(NNZ, dtype=np.float32)
    for c in range(N_CORES):
        o = np.asarray(LAST_RESULTS.results[c]["out"])  # [128, NT]
        flat = o.T.ravel()  # slot s = t*128 + p
        se = slot_maps[c]
        valid = se >= 0
        output[se[valid]] = flat[valid]
    return output


# revision 28
# speedup vs baseline: 1.2916x; 1.2916x over previous
"""Trainium2 Bass kernel for nn_Decoder (moe_routing, 4-species expert decoder).

Reference semantics (per species i, m = 4096 entries; only the first 512
decoded rows are ever read because decoded[bi, gi] indexes rows with *cell*
ids < 512):

    bi   = batch_idx[i*m:(i+1)*m]            # cell ids < 512
    gi   = gene_idx[i*m:(i+1)*m]
    comb = concat(z[i][:512], global_latent[bi[:512]])       # [512, 512]
    h1   = relu(comb @ W1[i] + b1[i])                        # [512, 1024]
    h2   = relu(h1 @ W2[i] + b2[i])                          # [512, 1024]
    out[e] = softplus(h2[bi[e]] . W3[i][:, gi[e]] + b3[i][gi[e]])

Sharding: expert-parallel x row-parallel.  Core c handles species c//2 and
MLP rows [256*(c%2), 256*(c%2+1)) plus every entry whose cell id falls in
that row range, so the two cores of a species split the MLP instead of
duplicating it.  The host pre-gathers everything indexable at input-prep
time: comb^T directly (z slice + global_latent[bi]), the W3 columns each
entry needs (w3gT, in [h, entry] layout), and b3[gi].  The per-entry dot
then runs on the PE: per 128-entry chunk, Q[e, c] = w3col_e . h2T[:, c]
(8 accumulating matmuls against the resident h2T), after which a one-hot
mask multiply + 128-wide reduce on the DVE picks Q[e, bi[e]].  h2 is kept
in [h, row] layout so both biases are per-partition activation biases.
Math in bf16 with f32 accumulation; no device gathers, no DRAM spills.
"""

import os
import sys

import numpy as np

for _p in ("/root/.axon_site/_ro/trn_rl_repo", "/opt/trn_rl_repo"):
    if os.path.isdir(_p) and _p not in sys.path:
        sys.path.append(_p)

import ml_dtypes

BF = ml_dtypes.bfloat16

N_SPECIES = 4
NNZ = 16384
N_CELLS = 512
L = 256          # latent
H = 1024         # hidden
G = 20000        # genes
M = NNZ // N_SPECIES   # 4096 entries per species
R = 512          # decoded rows actually used per species
RC = 256         # rows per core (half of R)
N_CORES = 8

_NC = {}              # CHT -> compiled Bass module
LAST_RESULTS = None   # BassKernelResults of the last run (for profiling)


def _build_nc(CHT):
    """CHT = 128-entry chunks per 128-row group (2 groups per core)."""
    from contextlib import ExitStack

    import concourse.bacc as bacc
    import concourse.mybir as mybir
    import concourse.tile as tile

    F32 = mybir.dt.float32
    BF16 = mybir.dt.bfloat16
    AF = mybir.ActivationFunctionType
    OP = mybir.AluOpType

    NT = 2 * CHT
    CB = 4 * RC          # combT cols in the bf16 pack
    OH0 = CB             # ohm offset in the bf16 pack

    nc = bacc.Bacc(None, target_bir_lowering=False)

    # pf32 packs b1s [128,8] | b2s [128,8] | b3g [128,NT].
    combT = nc.dram_tensor("combT", [128, CB], BF16, kind="ExternalInput")
    ohm = nc.dram_tensor("ohm", [128, NT * 128], BF16, kind="ExternalInput")
    pf32 = nc.dram_tensor("pf32", [128, 16 + NT], F32, kind="ExternalInput")
    w1r = nc.dram_tensor("w1r", [128, 8, 4, 128], BF16, kind="ExternalInput")
    w2r = nc.dram_tensor("w2r", [128, 8, 8, 128], BF16, kind="ExternalInput")
    w3gT = nc.dram_tensor("w3gT", [128, NT, 8, 128], BF16, kind="ExternalInput")
    out = nc.dram_tensor("out", [128, NT], F32, kind="ExternalOutput")

    with tile.TileContext(nc) as tc, ExitStack() as ctx:
        const = ctx.enter_context(tc.tile_pool(name="const", bufs=1))
        work = ctx.enter_context(tc.tile_pool(name="work", bufs=1))
        scrp = ctx.enter_context(tc.tile_pool(name="scr", bufs=2))
        pmm = ctx.enter_context(tc.tile_pool(name="pmm", bufs=2, space="PSUM"))
        pq = ctx.enter_context(tc.tile_pool(name="pq", bufs=4, space="PSUM"))

        # All loads ride the sync ring in strict consumption-priority order,
        # each piece its own tile so compute can start the moment its piece
        # lands: combT, w1 halves, biases, w2 halves, masks, then the W3
        # table streamed in dot order.  (The scalar ring stays free so relus
        # are never stuck behind DIRECT2D issue.)
        # Loads ride BOTH DMA rings (sync + scalar) so two SW-DGE engines
        # feed the 16 HW queues, alternating in consumption-priority order.
        # W3 pieces stream t-major so each 128-entry chunk's dot can run as
        # soon as its piece lands; the last pieces taper down to one chunk to
        # minimize the post-DMA tail.
        combT_s = const.tile([128, CB], BF16, tag="combT")
        nc.sync.dma_start(combT_s[:], combT[:])
        pf_s = const.tile([128, 16 + NT], F32, tag="pf")
        nc.sync.dma_start(pf_s[:], pf32[:])
        w1_t = []
        w1_dmas = []
        for h in range(4):
            w1_t.append(const.tile([128, 2, 4, 128], BF16, name=f"w1{h}", tag=f"w1{h}"))
            w1_dmas.append(nc.sync.dma_start(
                w1_t[h][:], w1r[:, 2 * h : 2 * (h + 1), :, :]))
        w2_t = []
        for h in range(4):
            w2_t.append(const.tile([128, 2, 8, 128], BF16, name=f"w2{h}", tag=f"w2{h}"))
            nc.sync.dma_start(w2_t[h][:], w2r[:, 2 * h : 2 * (h + 1), :, :])
        ohm_s = const.tile([128, NT * 128], BF16, tag="ohm")
        nc.sync.dma_start(ohm_s[:], ohm[:])
        pieces = []
        left = NT
        while left > 3:
            pieces.append(3)
            left -= 3
        pieces += [2, 1] if left == 3 else [left]
        assert sum(pieces) == NT

        # PE warmup while the weight DMA is in flight: throwaway matmuls keep
        # TensorE busy from the start so the activity monitor is in fast mode
        # (a cold PE runs at half rate) by the time h1 starts.
        wu = const.tile([128, RC], BF16, tag="wu")
        nc.gpsimd.memset(wu[:], 0)
        one1 = const.tile([128, 1], F32, tag="one1")
        nc.gpsimd.memset(one1[:], 1.0)
        for w in range(6):
            pw = pq.tile([128, 128], F32, tag="q")
            nc.tensor.matmul(pw[:], wu[:, 0:128], wu[:, 0:128], start=True,
                             stop=True)
        # Preload the Exp/Ln activation tables during the DMA wait so the
        # softplus tail doesn't eat a 1.3us ACT_TABLE_LOAD.
        wrm = work.tile([128, 2], F32, tag="wrm")
        nc.scalar.activation(wrm[:, 0:1], one1[:], AF.Exp)
        nc.scalar.activation(wrm[:, 1:2], one1[:], AF.Ln)

        # h1T[h, row] = relu(W1_mt.T @ combT + b1): per-partition bias.
        h1T = work.tile([128, 8, RC], BF16, tag="h1T")
        for mt in range(8):
            ps = pmm.tile([128, RC], F32, tag="ps")
            for kt in range(4):
                mm = nc.tensor.matmul(
                    ps[:],
                    w1_t[mt // 2][:, mt % 2, kt, :],
                    combT_s[:, kt * RC : (kt + 1) * RC],
                    start=(kt == 0),
                    stop=(kt == 3),
                )
            nc.scalar.activation(
                h1T[:, mt, :], ps[:], AF.Relu, bias=pf_s[:, mt : mt + 1]
            )

        # The W3 stream follows on the same sync ring, so the queue FIFOs
        # drain in exact consumption-priority order.
        w3_t = []   # per chunk t -> (tile, col offset)
        pi = 0
        for np_ in pieces:
            tw = const.tile([128, np_, 8, 128], BF16, name=f"w3p{pi}",
                            tag=f"w3p{pi}")
            nc.sync.dma_start(tw[:], w3gT[:, pi : pi + np_, :, :])
            for u in range(np_):
                w3_t.append((tw, u))
            pi += np_

        # h2T[h, row] = relu(W2_ht.T @ h1T + b2): per-partition bias.
        h2T = work.tile([128, 8, RC], BF16, tag="h2T")
        for ht in range(8):
            ps = pmm.tile([128, RC], F32, tag="ps")
            for k2 in range(8):
                nc.tensor.matmul(
                    ps[:],
                    w2_t[ht // 2][:, ht % 2, k2, :],
                    h1T[:, k2, :],
                    start=(k2 == 0),
                    stop=(k2 == 7),
                )
            nc.scalar.activation(
                h2T[:, ht, :], ps[:], AF.Relu, bias=pf_s[:, 8 + ht : 9 + ht]
            )

        # Per 128-entry chunk t (cell group g = t // CHT):
        #   Q[e, c] = sum_h w3gT[h, e] * h2T[h, c]     (8 matmuls, k-tiled)
        #   dots[e] = sum_c (Q[e,c] + b3[e]) * ohm[e,c] = Q[e, bi_e] + b3[e]
        # (each real mask row sums to 1, pad rows to 0) in one fused DVE op.
        dots = work.tile([128, NT], F32, tag="dots")
        for t in range(NT):
            g = t // CHT
            tw, u = w3_t[t]
            q = pq.tile([128, 128], F32, tag="q")
            for kt in range(8):
                nc.tensor.matmul(
                    q[:],
                    tw[:, u, kt, :],
                    h2T[:, kt, g * 128 : (g + 1) * 128],
                    start=(kt == 0),
                    stop=(kt == 7),
                )
            scr = scrp.tile([128, 128], BF16, tag="scr")
            nc.vector.scalar_tensor_tensor(
                out=scr[:],
                in0=q[:],
                scalar=pf_s[:, 16 + t : 17 + t],
                in1=ohm_s[:, t * 128 : (t + 1) * 128],
                op0=OP.add,
                op1=OP.mult,
                accum_out=dots[:, t : t + 1],
            )

        # softplus(x) = ln(e^x + 1) via the Exp and Ln activation tables,
        # the +1 folded into Ln's per-partition bias; both ops on Scalar.
        u = work.tile([128, NT], F32, tag="u")
        l = work.tile([128, NT], F32, tag="l")
        nc.scalar.activation(u[:], dots[:], AF.Exp)
        nc.scalar.activation(l[:], u[:], AF.Ln, bias=one1[:])
        nc.scalar.dma_start(out[:], l[:])

    nc.finalize()
    return nc


def _get_nc(CHT):
    if CHT not in _NC:
        _NC[CHT] = _build_nc(CHT)
    return _NC[CHT]


def _prep_core_inputs(c, CHT, batch_idx, gene_idx, global_latent, z, W1, b1,
                      W2, b2, W3, b3):
    """Build the device input map for core c plus the slot->global-entry map
    used to assemble the output (slot s = t*128 + p; -1 = padding)."""
    i, j = c // 2, c % 2
    NT = 2 * CHT
    CB = 4 * RC
    bi_sp = np.asarray(batch_idx[i * M : (i + 1) * M], dtype=np.int64)
    gi_sp = np.asarray(gene_idx[i * M : (i + 1) * M], dtype=np.int64)
    bi512 = np.asarray(batch_idx[i * M : i * M + R], dtype=np.int64)

    slot_entry = np.full(NT * 128, -1, dtype=np.int64)
    gi_slots = np.zeros(NT * 128, dtype=np.int64)
    b3_slots = np.zeros(NT * 128, dtype=np.float32)
    ohm = np.zeros((128, NT * 128), dtype=BF)  # [entry partition, t*128 + c]
    for g in range(2):
        eg = np.nonzero(bi_sp // 128 == 2 * j + g)[0]
        assert len(eg) <= CHT * 128
        ss = np.arange(len(eg))
        tt = g * CHT + ss // 128
        pp = ss % 128
        slots = tt * 128 + pp
        slot_entry[slots] = i * M + eg
        gi_slots[slots] = gi_sp[eg]
        b3_slots[slots] = b3[i][gi_sp[eg]]
        ohm[pp, tt * 128 + (bi_sp[eg] % 128)] = 1

    # comb^T for this core's rows: feature f x row r' (r = RC*j + r')
    rows = slice(RC * j, RC * (j + 1))
    comb = np.concatenate(
        [z[i][rows], global_latent[bi512[rows]]], axis=1)  # [RC, 2L]
    combT = comb.T.astype(BF)  # [512 feat, RC]
    # col layout: kt*RC + r' with feature f = kt*128 + p
    combT_pk = np.ascontiguousarray(
        combT.reshape(4, 128, RC).transpose(1, 0, 2).reshape(128, CB))

    pf32 = np.empty((128, 16 + NT), dtype=np.float32)
    pf32[:, 0:8] = b1[i].reshape(8, 128).T
    pf32[:, 8:16] = b2[i].reshape(8, 128).T
    pf32[:, 16:] = b3_slots.reshape(NT, 128).T

    # W3 columns for each slot in [h, entry] layout: w3gT[p, t, kt, q] =
    # W3[i][kt*128+p, gi_slots[t*128+q]]
    w3cols = W3[i][:, gi_slots].astype(BF)  # [H, NT*128]
    w3gT = np.ascontiguousarray(
        w3cols.reshape(8, 128, NT, 128).transpose(1, 2, 0, 3))

    in_map = {
        "combT": combT_pk,
        "ohm": ohm,
        "pf32": pf32,
        "w1r": np.ascontiguousarray(
            W1[i].reshape(4, 128, 8, 128).transpose(1, 2, 0, 3)).astype(BF),
        "w2r": np.ascontiguousarray(
            W2[i].reshape(8, 128, 8, 128).transpose(1, 2, 0, 3)).astype(BF),
        "w3gT": w3gT,
    }
    return in_map, slot_entry


def kernel(values, batch_idx, gene_idx, global_latent, z, W1, b1, W2, b2, W3,
           b3):
    global LAST_RESULTS
    from concourse.bass_utils import run_bass_kernel_spmd

    batch_idx = np.asarray(batch_idx)
    gene_idx = np.asarray(gene_idx)
    global_latent = np.asarray(global_latent, dtype=np.float32)
    z = np.asarray(z, dtype=np.float32)
    W1 = np.asarray(W1, dtype=np.float32)
    b1 = np.asarray(b1, dtype=np.float32)
    W2 = np.asarray(W2, dtype=np.float32)
    b2 = np.asarray(b2, dtype=np.float32)
    W3 = np.asarray(W3, dtype=np.float32)
    b3 = np.asarray(b3, dtype=np.float32)

    # Chunks per 128-row group: sized to the largest group so padding is <1
    # chunk; compiled kernels are cached per CHT.
    counts = np.bincount(np.asarray(batch_idx, dtype=np.int64) // 128
                         + 4 * (np.arange(NNZ) // M), minlength=16)
    CHT = int(-(-counts.max() // 128))
    nc = _get_nc(CHT)

    in_maps, slot_maps = [], []
    for c in range(N_CORES):
        im, se = _prep_core_inputs(c, CHT, batch_idx, gene_idx, global_latent,
                                   z, W1, b1, W2, b2, W3, b3)
        in_maps.append(im)
        slot_maps.append(se)

    LAST_RESULTS = run_bass_kernel_spmd(nc, in_maps, core_ids=list(range(N_CORES)))

    output = np.zeros(NNZ, dtype=np.float32)
    for c in range(N_CORES):
        o = np.asarray(LAST_RESULTS.results[c]["out"])  # [128, NT]
        flat = o.T.ravel()  # slot s = t*128 + p
        se = slot_maps[c]
        valid = se >= 0
        output[se[valid]] = flat[valid]
    return output


# revision 29
# speedup vs baseline: 1.3467x; 1.0427x over previous
"""Trainium2 Bass kernel for nn_Decoder (moe_routing, 4-species expert decoder).

Reference semantics (per species i, m = 4096 entries; only the first 512
decoded rows are ever read because decoded[bi, gi] indexes rows with *cell*
ids < 512):

    bi   = batch_idx[i*m:(i+1)*m]            # cell ids < 512
    gi   = gene_idx[i*m:(i+1)*m]
    comb = concat(z[i][:512], global_latent[bi[:512]])       # [512, 512]
    h1   = relu(comb @ W1[i] + b1[i])                        # [512, 1024]
    h2   = relu(h1 @ W2[i] + b2[i])                          # [512, 1024]
    out[e] = softplus(h2[bi[e]] . W3[i][:, gi[e]] + b3[i][gi[e]])

Sharding: expert-parallel x row-parallel.  Core c handles species c//2 and
MLP rows [256*(c%2), 256*(c%2+1)) plus every entry whose cell id falls in
that row range, so the two cores of a species split the MLP instead of
duplicating it.  The host pre-gathers everything indexable at input-prep
time: comb^T directly (z slice + global_latent[bi]), the W3 columns each
entry needs (w3gT, in [h, entry] layout), and b3[gi].  The per-entry dot
then runs on the PE: per 128-entry chunk, Q[e, c] = w3col_e . h2T[:, c]
(8 accumulating matmuls against the resident h2T), after which a one-hot
mask multiply + 128-wide reduce on the DVE picks Q[e, bi[e]].  h2 is kept
in [h, row] layout so both biases are per-partition activation biases.
Math in bf16 with f32 accumulation; no device gathers, no DRAM spills.
"""

import os
import sys

import numpy as np

for _p in ("/root/.axon_site/_ro/trn_rl_repo", "/opt/trn_rl_repo"):
    if os.path.isdir(_p) and _p not in sys.path:
        sys.path.append(_p)

import ml_dtypes

BF = ml_dtypes.bfloat16

N_SPECIES = 4
NNZ = 16384
N_CELLS = 512
L = 256          # latent
H = 1024         # hidden
G = 20000        # genes
M = NNZ // N_SPECIES   # 4096 entries per species
R = 512          # decoded rows actually used per species
RC = 256         # rows per core (half of R)
N_CORES = 8

_NC = {}              # CHT -> compiled Bass module
LAST_RESULTS = None   # BassKernelResults of the last run (for profiling)


def _build_nc(CHT):
    """CHT = 128-entry chunks per 128-row group (2 groups per core)."""
    from contextlib import ExitStack

    import concourse.bacc as bacc
    import concourse.mybir as mybir
    import concourse.tile as tile

    F32 = mybir.dt.float32
    BF16 = mybir.dt.bfloat16
    AF = mybir.ActivationFunctionType
    OP = mybir.AluOpType

    NT = 2 * CHT
    CB = 4 * RC          # combT cols in the bf16 pack
    OH0 = CB             # ohm offset in the bf16 pack

    nc = bacc.Bacc(None, target_bir_lowering=False)

    # pf32 packs b1s [128,8] | b2s [128,8] | b3g [128,NT].
    combT = nc.dram_tensor("combT", [128, CB], BF16, kind="ExternalInput")
    ohm = nc.dram_tensor("ohm", [128, NT * 128], BF16, kind="ExternalInput")
    pf32 = nc.dram_tensor("pf32", [128, 16 + NT], F32, kind="ExternalInput")
    w1r = nc.dram_tensor("w1r", [128, 8, 4, 128], BF16, kind="ExternalInput")
    w2r = nc.dram_tensor("w2r", [128, 8, 8, 128], BF16, kind="ExternalInput")
    w3gT = nc.dram_tensor("w3gT", [128, NT, 8, 128], BF16, kind="ExternalInput")
    out = nc.dram_tensor("out", [128, NT], F32, kind="ExternalOutput")

    with tile.TileContext(nc) as tc, ExitStack() as ctx:
        const = ctx.enter_context(tc.tile_pool(name="const", bufs=1))
        work = ctx.enter_context(tc.tile_pool(name="work", bufs=1))
        scrp = ctx.enter_context(tc.tile_pool(name="scr", bufs=2))
        pmm = ctx.enter_context(tc.tile_pool(name="pmm", bufs=2, space="PSUM"))
        pq = ctx.enter_context(tc.tile_pool(name="pq", bufs=4, space="PSUM"))

        # All loads ride the sync ring in strict consumption-priority order,
        # each piece its own tile so compute can start the moment its piece
        # lands: combT, w1 halves, biases, w2 halves, masks, then the W3
        # table streamed in dot order.  (The scalar ring stays free so relus
        # are never stuck behind DIRECT2D issue.)
        # Loads ride BOTH DMA rings (sync + scalar) so two SW-DGE engines
        # feed the 16 HW queues, alternating in consumption-priority order.
        # W3 pieces stream t-major so each 128-entry chunk's dot can run as
        # soon as its piece lands; the last pieces taper down to one chunk to
        # minimize the post-DMA tail.
        combT_s = const.tile([128, CB], BF16, tag="combT")
        nc.sync.dma_start(combT_s[:], combT[:])
        pf_s = const.tile([128, 16 + NT], F32, tag="pf")
        nc.sync.dma_start(pf_s[:], pf32[:])
        w1_t = []
        for h in range(2):
            w1_t.append(const.tile([128, 4, 4, 128], BF16, name=f"w1{h}", tag=f"w1{h}"))
            nc.sync.dma_start(w1_t[h][:], w1r[:, 4 * h : 4 * (h + 1), :, :])
        w2_t = []
        for h in range(2):
            w2_t.append(const.tile([128, 4, 8, 128], BF16, name=f"w2{h}", tag=f"w2{h}"))
            nc.sync.dma_start(w2_t[h][:], w2r[:, 4 * h : 4 * (h + 1), :, :])
        ohm_s = const.tile([128, NT * 128], BF16, tag="ohm")
        nc.sync.dma_start(ohm_s[:], ohm[:])
        pieces = []
        left = NT
        while left > 3:
            pieces.append(3)
            left -= 3
        pieces += [2, 1] if left == 3 else [left]
        assert sum(pieces) == NT

        # PE warmup while the weight DMA is in flight: throwaway matmuls keep
        # TensorE busy from the start so the activity monitor is in fast mode
        # (a cold PE runs at half rate) by the time h1 starts.
        wu = const.tile([128, RC], BF16, tag="wu")
        nc.gpsimd.memset(wu[:], 0)
        one1 = const.tile([128, 1], F32, tag="one1")
        nc.gpsimd.memset(one1[:], 1.0)
        for w in range(12):
            pw = pq.tile([128, 128], F32, tag="q")
            nc.tensor.matmul(pw[:], wu[:, 0:128], wu[:, 0:128], start=True,
                             stop=True)
        # Preload the Exp/Ln activation tables during the DMA wait so the
        # softplus tail doesn't eat a 1.3us ACT_TABLE_LOAD.
        wrm = work.tile([128, 2], F32, tag="wrm")
        nc.scalar.activation(wrm[:, 0:1], one1[:], AF.Exp)
        nc.scalar.activation(wrm[:, 1:2], one1[:], AF.Ln)

        # h1T[h, row] = relu(W1_mt.T @ combT + b1): per-partition bias.
        h1T = work.tile([128, 8, RC], BF16, tag="h1T")
        for mt in range(8):
            ps = pmm.tile([128, RC], F32, tag="ps")
            for kt in range(4):
                mm = nc.tensor.matmul(
                    ps[:],
                    w1_t[mt // 4][:, mt % 4, kt, :],
                    combT_s[:, kt * RC : (kt + 1) * RC],
                    start=(kt == 0),
                    stop=(kt == 3),
                )
            nc.scalar.activation(
                h1T[:, mt, :], ps[:], AF.Relu, bias=pf_s[:, mt : mt + 1]
            )

        # The W3 stream follows on the same sync ring, so the queue FIFOs
        # drain in exact consumption-priority order.
        w3_t = []   # per chunk t -> (tile, col offset)
        pi = 0
        for np_ in pieces:
            tw = const.tile([128, np_, 8, 128], BF16, name=f"w3p{pi}",
                            tag=f"w3p{pi}")
            nc.sync.dma_start(tw[:], w3gT[:, pi : pi + np_, :, :])
            for u in range(np_):
                w3_t.append((tw, u))
            pi += np_

        # h2T[h, row] = relu(W2_ht.T @ h1T + b2): per-partition bias.
        h2T = work.tile([128, 8, RC], BF16, tag="h2T")
        for ht in range(8):
            ps = pmm.tile([128, RC], F32, tag="ps")
            for k2 in range(8):
                nc.tensor.matmul(
                    ps[:],
                    w2_t[ht // 4][:, ht % 4, k2, :],
                    h1T[:, k2, :],
                    start=(k2 == 0),
                    stop=(k2 == 7),
                )
            nc.scalar.activation(
                h2T[:, ht, :], ps[:], AF.Relu, bias=pf_s[:, 8 + ht : 9 + ht]
            )

        # Per 128-entry chunk t (cell group g = t // CHT):
        #   Q[e, c] = sum_h w3gT[h, e] * h2T[h, c]     (8 matmuls, k-tiled)
        #   dots[e] = sum_c (Q[e,c] + b3[e]) * ohm[e,c] = Q[e, bi_e] + b3[e]
        # (each real mask row sums to 1, pad rows to 0) in one fused DVE op.
        dots = work.tile([128, NT], F32, tag="dots")
        for t in range(NT):
            g = t // CHT
            tw, u = w3_t[t]
            q = pq.tile([128, 128], F32, tag="q")
            for kt in range(8):
                nc.tensor.matmul(
                    q[:],
                    tw[:, u, kt, :],
                    h2T[:, kt, g * 128 : (g + 1) * 128],
                    start=(kt == 0),
                    stop=(kt == 7),
                )
            scr = scrp.tile([128, 128], BF16, tag="scr")
            nc.vector.scalar_tensor_tensor(
                out=scr[:],
                in0=q[:],
                scalar=pf_s[:, 16 + t : 17 + t],
                in1=ohm_s[:, t * 128 : (t + 1) * 128],
                op0=OP.add,
                op1=OP.mult,
                accum_out=dots[:, t : t + 1],
            )

        # softplus(x) = ln(e^x + 1) via the Exp and Ln activation tables,
        # the +1 folded into Ln's per-partition bias; both ops on Scalar.
        u = work.tile([128, NT], F32, tag="u")
        l = work.tile([128, NT], F32, tag="l")
        nc.scalar.activation(u[:], dots[:], AF.Exp)
        nc.scalar.activation(l[:], u[:], AF.Ln, bias=one1[:])
        nc.scalar.dma_start(out[:], l[:])

    nc.finalize()
    return nc


def _get_nc(CHT):
    if CHT not in _NC:
        _NC[CHT] = _build_nc(CHT)
    return _NC[CHT]


def _prep_core_inputs(c, CHT, batch_idx, gene_idx, global_latent, z, W1, b1,
                      W2, b2, W3, b3):
    """Build the device input map for core c plus the slot->global-entry map
    used to assemble the output (slot s = t*128 + p; -1 = padding)."""
    i, j = c // 2, c % 2
    NT = 2 * CHT
    CB = 4 * RC
    bi_sp = np.asarray(batch_idx[i * M : (i + 1) * M], dtype=np.int64)
    gi_sp = np.asarray(gene_idx[i * M : (i + 1) * M], dtype=np.int64)
    bi512 = np.asarray(batch_idx[i * M : i * M + R], dtype=np.int64)

    slot_entry = np.full(NT * 128, -1, dtype=np.int64)
    gi_slots = np.zeros(NT * 128, dtype=np.int64)
    b3_slots = np.zeros(NT * 128, dtype=np.float32)
    ohm = np.zeros((128, NT * 128), dtype=BF)  # [entry partition, t*128 + c]
    for g in range(2):
        eg = np.nonzero(bi_sp // 128 == 2 * j + g)[0]
        assert len(eg) <= CHT * 128
        ss = np.arange(len(eg))
        tt = g * CHT + ss // 128
        pp = ss % 128
        slots = tt * 128 + pp
        slot_entry[slots] = i * M + eg
        gi_slots[slots] = gi_sp[eg]
        b3_slots[slots] = b3[i][gi_sp[eg]]
        ohm[pp, tt * 128 + (bi_sp[eg] % 128)] = 1

    # comb^T for this core's rows: feature f x row r' (r = RC*j + r')
    rows = slice(RC * j, RC * (j + 1))
    comb = np.concatenate(
        [z[i][rows], global_latent[bi512[rows]]], axis=1)  # [RC, 2L]
    combT = comb.T.astype(BF)  # [512 feat, RC]
    # col layout: kt*RC + r' with feature f = kt*128 + p
    combT_pk = np.ascontiguousarray(
        combT.reshape(4, 128, RC).transpose(1, 0, 2).reshape(128, CB))

    pf32 = np.empty((128, 16 + NT), dtype=np.float32)
    pf32[:, 0:8] = b1[i].reshape(8, 128).T
    pf32[:, 8:16] = b2[i].reshape(8, 128).T
    pf32[:, 16:] = b3_slots.reshape(NT, 128).T

    # W3 columns for each slot in [h, entry] layout: w3gT[p, t, kt, q] =
    # W3[i][kt*128+p, gi_slots[t*128+q]]
    w3cols = W3[i][:, gi_slots].astype(BF)  # [H, NT*128]
    w3gT = np.ascontiguousarray(
        w3cols.reshape(8, 128, NT, 128).transpose(1, 2, 0, 3))

    in_map = {
        "combT": combT_pk,
        "ohm": ohm,
        "pf32": pf32,
        "w1r": np.ascontiguousarray(
            W1[i].reshape(4, 128, 8, 128).transpose(1, 2, 0, 3)).astype(BF),
        "w2r": np.ascontiguousarray(
            W2[i].reshape(8, 128, 8, 128).transpose(1, 2, 0, 3)).astype(BF),
        "w3gT": w3gT,
    }
    return in_map, slot_entry


def kernel(values, batch_idx, gene_idx, global_latent, z, W1, b1, W2, b2, W3,
           b3):
    global LAST_RESULTS
    from concourse.bass_utils import run_bass_kernel_spmd

    batch_idx = np.asarray(batch_idx)
    gene_idx = np.asarray(gene_idx)
    global_latent = np.asarray(global_latent, dtype=np.float32)
    z = np.asarray(z, dtype=np.float32)
    W1 = np.asarray(W1, dtype=np.float32)
    b1 = np.asarray(b1, dtype=np.float32)
    W2 = np.asarray(W2, dtype=np.float32)
    b2 = np.asarray(b2, dtype=np.float32)
    W3 = np.asarray(W3, dtype=np.float32)
    b3 = np.asarray(b3, dtype=np.float32)

    # Chunks per 128-row group: sized to the largest group so padding is <1
    # chunk; compiled kernels are cached per CHT.
    counts = np.bincount(np.asarray(batch_idx, dtype=np.int64) // 128
                         + 4 * (np.arange(NNZ) // M), minlength=16)
    CHT = int(-(-counts.max() // 128))
    nc = _get_nc(CHT)

    in_maps, slot_maps = [], []
    for c in range(N_CORES):
        im, se = _prep_core_inputs(c, CHT, batch_idx, gene_idx, global_latent,
                                   z, W1, b1, W2, b2, W3, b3)
        in_maps.append(im)
        slot_maps.append(se)

    LAST_RESULTS = run_bass_kernel_spmd(nc, in_maps, core_ids=list(range(N_CORES)))

    output = np.zeros(NNZ, dtype=np.float32)
    for c in range(N_CORES):
        o = np.asarray(LAST_RESULTS.results[c]["out"])  # [128, NT]
        flat = o.T.ravel()  # slot s = t*128 + p
        se = slot_maps[c]
        valid = se >= 0
        output[se[valid]] = flat[valid]
    return output
